# revision 1
# baseline (speedup 1.0000x reference)
"""Trainium2 Bass kernel for GAT-style attention score computation.

Math (see reference):
    s_src = X @ a[:F];  s_dst = X @ a[F:]
    e[i, j] = leaky_relu(s_src[i] + s_dst[j], alpha=0.2)

Sharding over 8 NeuronCores: row-shard X (1024 rows/core). Each core
computes its local s_src/s_dst slices, AllGathers s_dst (8192 floats),
and emits its [1024, 8192] row block of e.

Per-core dataflow:
  - X shard loaded as one SBUF tile [128, 2048] (partition p holds rows
    8p..8p+7 of the local shard).
  - a_src/a_dst broadcast to [128, 256] via a ones[1,128] matmul.
  - s_src/s_dst per sub-row via fused tensor_tensor_reduce (DVE).
  - s_dst [128, 8] -> DRAM (row order) -> AllGather -> [8192] global.
  - Gathered s_dst broadcast to a [128, 8192] SBUF tile (d_bcast) via 16
    ones-matmuls (PSUM) + copies.
  - For each of the 8 sub-rows t: ONE ScalarEngine activation
    out = Lrelu(d_bcast + s_src[:, t], alpha=0.2)  -> [128, 8192] tile,
    DMA'd to the interleaved output rows (8p + t).

The kernel is output-write bound: 32 MB/core of f32 at ~360 GB/s.
"""

import numpy as np

N = 8192
F = 256
NCORES = 8
ROWS = N // NCORES          # 1024 rows per core
P = 128                     # partitions
C = ROWS // P               # 8 sub-rows per partition
ALPHA = 0.2

_CACHE = {}


def _build():
    import concourse.bacc as bacc
    import concourse.bass as bass
    import concourse.tile as tile
    from concourse import mybir

    fp32 = mybir.dt.float32

    nc = bacc.Bacc(
        "TRN2",
        target_bir_lowering=False,
        debug=False,
        num_devices=NCORES,
    )

    x_dram = nc.dram_tensor("feature_matrix", [ROWS, F], fp32, kind="ExternalInput")
    av_dram = nc.dram_tensor("attention_vector", [2 * F, 1], fp32, kind="ExternalInput")
    out_dram = nc.dram_tensor("out", [ROWS, N], fp32, kind="ExternalOutput")

    with tile.TileContext(nc) as tc:
        with (
            tc.tile_pool(name="const", bufs=1) as const_pool,
            tc.tile_pool(name="work", bufs=2) as work_pool,
            tc.tile_pool(name="dbc", bufs=1) as dbc_pool,
            tc.tile_pool(name="outp", bufs=3) as out_pool,
            tc.tile_pool(name="psum", bufs=4, space=bass.MemorySpace.PSUM) as psum_pool,
            tc.tile_pool(name="dram", bufs=1, space="DRAM") as dram_pool,
        ):
            # ---- load inputs ----
            x_sb = const_pool.tile([P, C * F], fp32)   # row 8p+c at [p, c*F:(c+1)*F]
            nc.sync.dma_start(x_sb[:], x_dram.ap().rearrange("(p c) f -> p (c f)", p=P))

            av_sb = const_pool.tile([1, 2 * F], fp32)
            nc.sync.dma_start(av_sb[:], av_dram.ap().rearrange("f one -> one f"))

            ones_sb = const_pool.tile([1, P], fp32)
            nc.vector.memset(ones_sb[:], 1.0)

            # ---- broadcast a_src / a_dst across partitions: ones^T @ row ----
            a_ps = psum_pool.tile([P, 2 * F], fp32)
            nc.tensor.matmul(a_ps[:], ones_sb[:], av_sb[:], start=True, stop=True)
            ab_sb = const_pool.tile([P, 2 * F], fp32)  # [:, :F]=a_src, [:, F:]=a_dst
            nc.vector.tensor_copy(ab_sb[:], a_ps[:])

            # ---- local matvecs: s_dst first (gates the collective) ----
            s_dst = const_pool.tile([P, C], fp32)
            s_src = const_pool.tile([P, C], fp32)

            def matvec(dst_col, a_slice, c):
                scratch = work_pool.tile([P, F], fp32, tag="mv_scratch")
                nc.vector.tensor_tensor(
                    scratch[:], x_sb[:, c * F:(c + 1) * F], a_slice,
                    op=mybir.AluOpType.mult,
                )
                nc.vector.tensor_reduce(
                    dst_col, scratch[:],
                    axis=mybir.AxisListType.X, op=mybir.AluOpType.add,
                )

            for c in range(C):
                matvec(s_dst[:, c:c + 1], ab_sb[:, F:], c)

            # s_dst -> DRAM in global row order (flat index p*C + c)
            cc_in = dram_pool.tile([P, C], fp32)
            nc.sync.dma_start(cc_in[:], s_dst[:])

            cc_out = dram_pool.tile([2 * C, N // (2 * C)], fp32)  # [16, 512] = 8192
            nc.gpsimd.collective_compute(
                "AllGather",
                mybir.AluOpType.bypass,
                replica_groups=[list(range(NCORES))],
                ins=[cc_in[:].opt()],
                outs=[cc_out[:].opt()],
            )

            # s_src while the collective is in flight
            for c in range(C):
                matvec(s_src[:, c:c + 1], ab_sb[:, :F], c)

            # ---- gathered s_dst -> SBUF [1, 8192], then broadcast to all partitions
            gath_sb = const_pool.tile([1, N], fp32)
            nc.sync.dma_start(gath_sb[:], cc_out[:].rearrange("a b -> (a b)").unsqueeze(0))

            d_bcast = dbc_pool.tile([P, N], fp32)
            NB = 512
            for k in range(N // NB):
                d_ps = psum_pool.tile([P, NB], fp32, tag="d_ps")
                nc.tensor.matmul(
                    d_ps[:], ones_sb[:], gath_sb[0:1, k * NB:(k + 1) * NB],
                    start=True, stop=True,
                )
                nc.vector.tensor_copy(d_bcast[:, k * NB:(k + 1) * NB], d_ps[:])

            # ---- main loop: one fused Lrelu per sub-row, then stream out ----
            out_view = out_dram.ap().rearrange("(p c) n -> p c n", p=P)
            for t in range(C):
                o = out_pool.tile([P, N], fp32)
                nc.scalar.activation(
                    o[:],
                    d_bcast[:],
                    mybir.ActivationFunctionType.Prelu,
                    bias=s_src[:, t:t + 1],
                    scale=1.0,
                    alpha=ALPHA,
                )
                nc.sync.dma_start(out_view[:, t, :], o[:])

    nc.compile()
    return nc


def _get_nc():
    if "nc" not in _CACHE:
        _CACHE["nc"] = _build()
    return _CACHE["nc"]


def kernel(feature_matrix: np.ndarray, attention_vector: np.ndarray) -> np.ndarray:
    from concourse.bass_utils import run_bass_kernel_spmd

    feature_matrix = np.ascontiguousarray(feature_matrix, dtype=np.float32)
    attention_vector = np.ascontiguousarray(attention_vector, dtype=np.float32)

    nc = _get_nc()
    in_maps = [
        {
            "feature_matrix": feature_matrix[c * ROWS:(c + 1) * ROWS],
            "attention_vector": attention_vector,
        }
        for c in range(NCORES)
    ]
    res = run_bass_kernel_spmd(nc, in_maps, core_ids=list(range(NCORES)))
    return np.concatenate([res.results[c]["out"] for c in range(NCORES)], axis=0)

